# revision 1
# baseline (speedup 1.0000x reference)
"""Trainium2 Bass kernel for the 2-layer LSTMCell model.

Model (per timestep t, torch.nn.LSTMCell semantics, gates (i,f,g,o)):
    h0,c0 = LSTMCell(x_t, (h0,c0))   # D_IN=16  -> H1=100
    h1,c1 = LSTMCell(h0, (h1,c1))    # H1=100 -> H2=50
    y = h1_final @ W_fc.T + b_fc     # [B, 1]

Strategy (8 NeuronCores, data parallel over batch):
  - Each core handles B_local = 256 rows as 2 chunks of 128.
  - States are kept in transposed layout [H, B] in SBUF so they can feed the
    PE matmul as the stationary operand (contraction on partitions).
  - Gate matmul (mapping: batch on PSUM partitions, gates on free dim):
        gates0[128b, 400] = lhsT([h0T; ones; xT]).T @ W0aug[117, 400]
    with biases folded in via a constant ones-row that arrives with the x DMA.
  - Gate order is permuted to (i, f, o, g) so one Sigmoid instruction covers
    i,f,o contiguously and one Tanh covers g.
  - Elementwise work for both chunks is merged into single ACT/DVE
    instructions using 2-level access patterns across PSUM banks.
  - h_new is transposed back to [H, B] with PE transposes + a copy into the
    next step's stationary tile.
"""

import sys

import ml_dtypes
import numpy as np

BF = ml_dtypes.bfloat16

sys.path.insert(0, "/opt/trn_rl_repo")

import concourse.bacc as bacc
import concourse.bass as bass
import concourse.mybir as mybir
from concourse.tile import TileContext

F32 = mybir.dt.float32
F32R = mybir.dt.float32r
BF16 = mybir.dt.bfloat16
Act = mybir.ActivationFunctionType

B, T, D_IN = 2048, 2048, 16
H1, H2 = 100, 50
N_CORES = 8
B_LOCAL = B // N_CORES        # 256
NCH = 2                       # chunks of 128 per core

LAST_EXEC_NS = None
LAST_RESULTS = None

# ---------------------------------------------------------------- kernel build


def build_nc(t_steps=T):
    nc = bacc.Bacc("TRN2", target_bir_lowering=False)
    xt_d = nc.dram_tensor("xt", [t_steps + 1, 17, 256], F32R, kind="ExternalInput").ap()
    # all constants in one blob -> one DMA -> one sem for every weight use
    cb_d = nc.dram_tensor("cblob", [128, 1297], F32R, kind="ExternalInput").ap()
    a0_d = nc.dram_tensor("a0", [117, 256], F32R, kind="ExternalInput").ap()
    y_d = nc.dram_tensor("y", [256, 1], F32, kind="ExternalOutput").ap()

    with TileContext(nc) as tc:
        with (
            tc.tile_pool(name="consts", bufs=1) as cp,
            tc.tile_pool(name="apool", bufs=8) as apool,
            tc.tile_pool(name="bpool", bufs=8) as bpool,
            tc.tile_pool(name="ew", bufs=3) as ew,
            tc.tile_pool(name="gps", bufs=2, space="PSUM") as gps,
            tc.tile_pool(name="tps", bufs=1, space="PSUM") as tps,
        ):
            cb = cp.tile([128, 1041], F32R)  # zeros tail of cblob stays in DRAM
            nc.sync.dma_start(cb, cb_d[:, 0:1041])
            w0 = cb[0:117, 0:400]
            wih1 = cb[0:101, 400:656]
            whh1 = cb[0:50, 656:912]
            wfc = cb[0:51, 912:913]
            ident = cb[0:128, 913:1041]

            # initial state: A(0) fully from one DMA (zeros + ones row + x0);
            # the rest zeroed on DVE (one sem, later subsumed)
            A = apool.tile([117, 256], F32R, tag="A")
            nc.sync.dma_start(A, a0_d)
            Btile = bpool.tile([50, 256], F32R, tag="B")
            nc.sync.dma_start(Btile, cb_d[0:50, 1041:1297])
            c0 = ew.tile([128, 200], F32, tag="c0")
            nc.vector.memset(c0[:, :], 0.0)
            c1 = ew.tile([128, 100], F32, tag="c1")
            nc.vector.memset(c1[:, :], 0.0)

            for t in range(t_steps):
                # ---- layer 0 gates: [128b, 400] per chunk, 2 chunks in 2 banks
                g0 = gps.tile([128, 1024], F32, tag="g0")
                for c in range(NCH):
                    nc.tensor.matmul(
                        g0[:, c * 512 : c * 512 + 400],
                        A[:, c * 128 : (c + 1) * 128],
                        w0,
                        start=True,
                        stop=True,
                    )
                g0v = g0.rearrange("p (c f) -> p c f", c=2)  # [128, 2, 512]

                s0 = ew.tile([128, 600], F32, tag="s0")      # sig(i,f,o) both chunks
                s0v = s0.rearrange("p (c f) -> p c f", c=2)
                nc.scalar.activation(s0v, g0v[:, :, 0:300], Act.Sigmoid)
                tg0 = ew.tile([128, 200], F32, tag="tg0")    # tanh(g)
                tg0v = tg0.rearrange("p (c f) -> p c f", c=2)
                nc.scalar.activation(tg0v, g0v[:, :, 300:400], Act.Tanh)

                c0v = c0.rearrange("p (c f) -> p c f", c=2)
                m1 = ew.tile([128, 200], F32, tag="m1")
                m1v = m1.rearrange("p (c f) -> p c f", c=2)
                nc.vector.tensor_mul(m1v, s0v[:, :, 0:100], tg0v)
                m2 = ew.tile([128, 200], F32, tag="m2")
                m2v = m2.rearrange("p (c f) -> p c f", c=2)
                nc.vector.tensor_mul(m2v, s0v[:, :, 100:200], c0v)
                c0n = ew.tile([128, 200], F32, tag="c0")
                nc.vector.tensor_add(c0n, m1, m2)
                thc0 = ew.tile([128, 200], F32, tag="thc0")
                nc.scalar.activation(thc0, c0n, Act.Tanh)
                h0 = ew.tile([128, 200], F32R, tag="h0")
                h0v = h0.rearrange("p (c f) -> p c f", c=2)
                thc0v = thc0.rearrange("p (c f) -> p c f", c=2)
                nc.vector.tensor_mul(h0v, s0v[:, :, 200:300], thc0v)

                # ---- recycle h0 into the next stationary tile (transposed)
                An = apool.tile([117, 256], F32R, tag="A")
                nc.sync.dma_start(An[100:117, :], xt_d[t + 1])
                t0 = tps.tile([100, 256], F32R, tag="t0")
                for c in range(NCH):
                    nc.tensor.transpose(
                        t0[:, c * 128 : (c + 1) * 128],
                        h0[:, c * 100 : (c + 1) * 100],
                        ident,
                    )
                nc.vector.tensor_copy(An[0:100, :], t0)

                # ---- layer 1 gates: [128b, 200] per chunk, 1 bank
                g1 = gps.tile([128, 512], F32, tag="g1")
                for c in range(NCH):
                    nc.tensor.matmul(
                        g1[:, c * 256 : (c + 1) * 256],
                        An[0:101, c * 128 : (c + 1) * 128],
                        wih1,
                        start=True,
                        stop=False,
                    )
                    nc.tensor.matmul(
                        g1[:, c * 256 : (c + 1) * 256],
                        Btile[0:50, c * 128 : (c + 1) * 128],
                        whh1,
                        start=False,
                        stop=True,
                    )
                g1v = g1.rearrange("p (c f) -> p c f", c=2)  # [128, 2, 256]

                s1 = ew.tile([128, 300], F32, tag="s1")
                s1v = s1.rearrange("p (c f) -> p c f", c=2)
                nc.scalar.activation(s1v, g1v[:, :, 0:150], Act.Sigmoid)
                tg1 = ew.tile([128, 100], F32, tag="tg1")
                tg1v = tg1.rearrange("p (c f) -> p c f", c=2)
                nc.scalar.activation(tg1v, g1v[:, :, 150:200], Act.Tanh)

                c1v = c1.rearrange("p (c f) -> p c f", c=2)
                m3 = ew.tile([128, 100], F32, tag="m3")
                m3v = m3.rearrange("p (c f) -> p c f", c=2)
                nc.vector.tensor_mul(m3v, s1v[:, :, 0:50], tg1v)
                m4 = ew.tile([128, 100], F32, tag="m4")
                m4v = m4.rearrange("p (c f) -> p c f", c=2)
                nc.vector.tensor_mul(m4v, s1v[:, :, 50:100], c1v)
                c1n = ew.tile([128, 100], F32, tag="c1")
                nc.vector.tensor_add(c1n, m3, m4)
                thc1 = ew.tile([128, 100], F32, tag="thc1")
                nc.scalar.activation(thc1, c1n, Act.Tanh)
                h1 = ew.tile([128, 100], F32R, tag="h1")
                h1v = h1.rearrange("p (c f) -> p c f", c=2)
                thc1v = thc1.rearrange("p (c f) -> p c f", c=2)
                nc.vector.tensor_mul(h1v, s1v[:, :, 100:150], thc1v)

                Bn = bpool.tile([50, 256], F32R, tag="B")
                t1 = tps.tile([50, 256], F32R, tag="t1")
                for c in range(NCH):
                    nc.tensor.transpose(
                        t1[:, c * 128 : (c + 1) * 128],
                        h1[:, c * 50 : (c + 1) * 50],
                        ident,
                    )
                nc.scalar.copy(Bn, t1)

                A, Btile, c0, c1 = An, Bn, c0n, c1n

            # ---- final projection y = h1 @ W_fc.T + b_fc
            fin = ew.tile([51, 256], F32R, tag="fin")
            nc.vector.tensor_copy(fin[0:50, :], Btile)
            nc.sync.dma_start(fin[50:51, :], xt_d[t_steps, 0:1, :])
            yp = gps.tile([128, 2], F32, tag="g1")
            for c in range(NCH):
                nc.tensor.matmul(
                    yp[:, c : c + 1],
                    fin[:, c * 128 : (c + 1) * 128].bitcast(F32),
                    wfc.bitcast(F32),
                    start=True,
                    stop=True,
                )
            ysb = ew.tile([128, 2], F32, tag="ysb")
            nc.scalar.copy(ysb, yp)
            yv = y_d.rearrange("(c p) o -> c p o", c=2)
            for c in range(NCH):
                nc.sync.dma_start(yv[c], ysb[:, c : c + 1])
    return nc


# ---------------------------------------------------------------- host prep


def _gate_perm_rows(w, h):
    """Reorder gate rows (i,f,g,o) -> (i,f,o,g)."""
    return np.concatenate([w[0:h], w[h : 2 * h], w[3 * h : 4 * h], w[2 * h : 3 * h]], axis=0)


def prep_weights(W_ih0, W_hh0, b_ih0, b_hh0, W_ih1, W_hh1, b_ih1, b_hh1, W_fc, b_fc):
    """Pack all constants into one [128, 929] blob (single DMA)."""
    f32 = np.float32
    cb = np.zeros((128, 1297), f32)
    cb[0:100, 0:400] = _gate_perm_rows(np.asarray(W_hh0), H1).T
    cb[100, 0:400] = _gate_perm_rows(np.asarray(b_ih0 + b_hh0)[:, None], H1)[:, 0]
    cb[101:117, 0:400] = _gate_perm_rows(np.asarray(W_ih0), H1).T
    cb[0:100, 400:600] = _gate_perm_rows(np.asarray(W_ih1), H2).T
    cb[100, 400:600] = _gate_perm_rows(np.asarray(b_ih1 + b_hh1)[:, None], H2)[:, 0]
    cb[0:50, 656:856] = _gate_perm_rows(np.asarray(W_hh1), H2).T
    cb[0:50, 912] = np.asarray(W_fc)[0]
    cb[50, 912] = np.asarray(b_fc)[0]
    cb[:, 913:1041] = np.eye(128, dtype=f32)
    return cb


def prep_x_core(x_core, t_steps):
    """x_core [256, T, 16] -> [T+1, 17, 256] with ones row at index 0."""
    xt = np.empty((t_steps + 1, 17, 256), np.float32)
    xt[:, 0, :] = 1.0
    xt[:t_steps, 1:17, :] = np.asarray(x_core).transpose(1, 2, 0)
    xt[t_steps, 1:17, :] = 0.0
    return xt


_RUNNER_CACHE = {}


def _get_runner(t_steps):
    """Compile once; return fn(concat_inputs: dict name->global np array) -> y
    plus a bench fn that re-executes on device-resident inputs."""
    if t_steps in _RUNNER_CACHE:
        return _RUNNER_CACHE[t_steps]

    import jax
    from jax.experimental.shard_map import shard_map
    from jax.sharding import Mesh, NamedSharding, PartitionSpec

    from concourse import bass2jax

    bass2jax.install_neuronx_cc_hook()
    nc = build_nc(t_steps)
    if not nc.is_finalized():
        nc.finalize()
    global _LAST_NC
    _LAST_NC = nc

    partition_name = (
        nc.partition_id_tensor.name if nc.partition_id_tensor else None
    )
    in_names = []
    out_names = []
    out_avals = []
    zero_outs = []
    for alloc in nc.m.functions[0].allocations:
        if not isinstance(alloc, mybir.MemoryLocationSet):
            continue
        name = alloc.memorylocations[0].name
        if alloc.kind == "ExternalInput":
            if name == partition_name:
                continue
            in_names.append(name)
        elif alloc.kind == "ExternalOutput":
            out_names.append(name)
            shape = tuple(alloc.tensor_shape)
            dtype = mybir.dt.np(alloc.dtype)
            out_avals.append(jax.core.ShapedArray(shape, dtype))
            zero_outs.append(np.zeros(shape, dtype))
    n_params = len(in_names)
    all_in_names = in_names + out_names
    if partition_name is not None:
        all_in_names = all_in_names + [partition_name]

    def _body(*args):
        operands = list(args)
        if partition_name is not None:
            operands.append(bass2jax.partition_id_tensor())
        outs = bass2jax._bass_exec_p.bind(
            *operands,
            out_avals=tuple(out_avals),
            in_names=tuple(all_in_names),
            out_names=tuple(out_names),
            lowering_input_output_aliases=(),
            sim_require_finite=True,
            sim_require_nnan=True,
            nc=nc,
        )
        return tuple(outs)

    devices = jax.devices()[:N_CORES]
    mesh = Mesh(np.asarray(devices), ("core",))
    spec = PartitionSpec("core")
    in_specs = (spec,) * (n_params + len(out_names))
    out_specs = (spec,) * len(out_names)
    sharded = jax.jit(
        shard_map(_body, mesh=mesh, in_specs=in_specs, out_specs=out_specs,
                  check_rep=False),
        keep_unused=True,
    )
    sharding = NamedSharding(mesh, spec)

    def run(concat_inputs, n_bench=0):
        import time as _time

        args = [jax.device_put(concat_inputs[n], sharding) for n in in_names]
        args += [jax.device_put(
            np.zeros((N_CORES * z.shape[0], *z.shape[1:]), z.dtype), sharding)
            for z in zero_outs]
        outs = jax.block_until_ready(sharded(*args))
        bench_ns = None
        if n_bench:
            times = []
            for _ in range(n_bench):
                t0 = _time.perf_counter()
                jax.block_until_ready(sharded(*args))
                times.append(_time.perf_counter() - t0)
            bench_ns = int(min(times) * 1e9)
        y = np.asarray(outs[out_names.index("y")])
        return y, bench_ns

    _RUNNER_CACHE[t_steps] = run
    return run


def make_inputs(x, W_ih0, W_hh0, b_ih0, b_hh0, W_ih1, W_hh1, b_ih1, b_hh1,
                W_fc, b_fc):
    x = np.asarray(x, dtype=np.float32)
    t_steps = x.shape[1]
    cb = prep_weights(
        W_ih0, W_hh0, b_ih0, b_hh0, W_ih1, W_hh1, b_ih1, b_hh1, W_fc, b_fc
    )
    xt_all = np.empty((N_CORES * (t_steps + 1), 17, 256), np.float32)
    a0_all = np.zeros((N_CORES * 117, 256), np.float32)
    for core in range(N_CORES):
        xc = x[core * B_LOCAL : (core + 1) * B_LOCAL]
        xt = prep_x_core(xc, t_steps)
        xt_all[core * (t_steps + 1) : (core + 1) * (t_steps + 1)] = xt
        a0_all[core * 117 + 100 : (core + 1) * 117] = xt[0]
    reps = lambda a: np.concatenate([a] * N_CORES, axis=0)
    return t_steps, {
        "xt": xt_all,
        "cblob": reps(cb),
        "a0": a0_all,
    }


def kernel(x, W_ih0, W_hh0, b_ih0, b_hh0, W_ih1, W_hh1, b_ih1, b_hh1, W_fc, b_fc,
           n_bench=0):
    global LAST_EXEC_NS
    t_steps, concat_inputs = make_inputs(
        x, W_ih0, W_hh0, b_ih0, b_hh0, W_ih1, W_hh1, b_ih1, b_hh1, W_fc, b_fc
    )
    run = _get_runner(t_steps)
    y, bench_ns = run(concat_inputs, n_bench=n_bench)
    if bench_ns is not None:
        LAST_EXEC_NS = bench_ns
    return y.astype(np.float32)



# revision 4
# speedup vs baseline: 6.5093x; 6.5093x over previous
"""Trainium2 Bass kernel for the 2-layer LSTMCell model (v5).

Model (per timestep t, torch.nn.LSTMCell semantics, gates (i,f,g,o)):
    h0,c0 = LSTMCell(x_t, (h0,c0))   # D_IN=16  -> H1=100
    h1,c1 = LSTMCell(h0, (h1,c1))    # H1=100 -> H2=50
    y = h1_final @ W_fc.T + b_fc     # [B, 1]

Strategy (8 NeuronCores, data parallel over batch, 256 rows/core as
2 chunks of 128):

  - All matmul operands bf16; cell states bf16; PSUM f32 (except bf16
    transpose outputs).
  - Gates i,f,g arrive batch-major ([128b, gates]) from per-chunk matmuls
    with the recurrent state A=[h0;1;x_t] ([117, 256]) as stationary.
  - tanh(g) is computed as 2*sigmoid(2g)-1: the g-columns of the weights
    are pre-scaled by 2 on the host so ONE Sigmoid instruction covers
    i,f,g of both chunks; the 2x-1 fixup is a DVE tensor_scalar.
  - The o gate is computed TRANSPOSED ([100h, 256b]) by a separate matmul
    with the weight block as stationary. c_new is transposed on the PE,
    tanh'd on ACT, and h = sig(o) * tanh(c) is then a [100,256] DVE mul
    that writes the next step's stationary tile directly -- no
    PSUM->SBUF copy on the recurrence path.
  - Layer 1 follows the same pattern at half width.
  - x_t rows (plus a ones row for biases) are DMA'd into the A tiles
    4 steps ahead of use, off the critical path.
  - Optional PE "heater" matmuls keep the tensor engine out of its low
    p-state during the elementwise phases of each step.
"""

import sys

import ml_dtypes
import numpy as np

BF = ml_dtypes.bfloat16

sys.path.insert(0, "/opt/trn_rl_repo")

import concourse.bacc as bacc
import concourse.bass as bass
import concourse.mybir as mybir
from concourse.tile import TileContext

F32 = mybir.dt.float32
BF16 = mybir.dt.bfloat16
Act = mybir.ActivationFunctionType
Alu = mybir.AluOpType

B, T, D_IN = 2048, 2048, 16
H1, H2 = 100, 50
N_CORES = 8
B_LOCAL = B // N_CORES        # 256
NCH = 2                       # chunks of 128 per core

# PE heater: dummy matmuls emitted (1) before the c transposes and
# (2) after them, to keep the PE clock ramped while DVE/ACT work.
HEAT1 = 0
HEAT2 = 0
HEAT_COLS = 200

PREFETCH = 4                  # steps of x-DMA prefetch into A tiles

LAST_EXEC_NS = None

# ---------------------------------------------------------------- kernel build


def build_nc(t_steps=T):
    nc = bacc.Bacc("TRN2", target_bir_lowering=False)
    xt_d = nc.dram_tensor("xt", [t_steps + 1, 17, 256], BF16, kind="ExternalInput").ap()
    cb_d = nc.dram_tensor("cblob", [128, 929], BF16, kind="ExternalInput").ap()
    a0_d = nc.dram_tensor("a0", [117, 256], BF16, kind="ExternalInput").ap()
    y_d = nc.dram_tensor("y", [1, 256], F32, kind="ExternalOutput").ap()

    with TileContext(nc) as tc:
        with (
            tc.tile_pool(name="consts", bufs=1) as cp,
            tc.tile_pool(name="apool", bufs=8) as apool,
            tc.tile_pool(name="bpool", bufs=4) as bpool,
            tc.tile_pool(name="ew", bufs=2) as ew,
            tc.tile_pool(name="g0ps", bufs=1, space="PSUM") as g0pool,
            tc.tile_pool(name="g1ps", bufs=1, space="PSUM") as g1pool,
            tc.tile_pool(name="ops", bufs=1, space="PSUM") as opool,
            tc.tile_pool(name="tps", bufs=1, space="PSUM") as tpool,
        ):
            cb = cp.tile([128, 929], BF16)
            nc.sync.dma_start(cb, cb_d)
            w0 = cb[0:117, 0:400]        # cols: i,f,2g (0:300) | o (300:400)
            wih1 = cb[0:101, 400:600]    # cols: i,f,2g (0:150) | o (150:200)
            whh1 = cb[0:50, 600:800]
            wfcb = cb[0:51, 800:801]     # rows 0:50 = W_fc, row 50 = b_fc
            ident = cb[0:128, 801:929]

            # initial state
            A_q = []
            a = apool.tile([117, 256], BF16, tag="A")
            nc.sync.dma_start(a, a0_d)
            A_q.append(a)
            for k in range(1, PREFETCH + 1):
                a = apool.tile([117, 256], BF16, tag="A")
                nc.sync.dma_start(a[100:117, :], xt_d[min(k, t_steps)])
                A_q.append(a)
            Btile = bpool.tile([50, 256], BF16, tag="B")
            nc.vector.memset(Btile[:, :], 0.0)
            c0 = ew.tile([128, 200], BF16, tag="c0")
            nc.vector.memset(c0[:, :], 0.0)
            c1 = ew.tile([128, 100], BF16, tag="c1")
            nc.vector.memset(c1[:, :], 0.0)

            def emit_g0_sig(t):
                """g0/oT matmuls + sigmoids for step t (reads A(t))."""
                A = A_q[t]
                g0 = g0pool.tile([128, 1024], F32, tag="g0")
                for c in range(NCH):
                    nc.tensor.matmul(
                        g0[:, c * 512 : c * 512 + 300],
                        A[:, c * 128 : (c + 1) * 128],
                        w0[:, 0:300],
                        start=True, stop=True,
                    )
                po = opool.tile([100, 256], F32, tag="po")
                nc.tensor.matmul(po, w0[:, 300:400], A, start=True, stop=True)
                g0v = g0.rearrange("p (c f) -> p c f", c=2)   # [128, 2, 512]
                S = ew.tile([128, 600], BF16, tag="S")
                Sv = S.rearrange("p (c f) -> p c f", c=2)     # [128, 2, 300]
                nc.scalar.activation(Sv, g0v[:, :, 0:300], Act.Sigmoid)
                SoT = ew.tile([100, 256], BF16, tag="SoT")
                nc.scalar.activation(SoT, po, Act.Sigmoid)
                return g0, S, SoT

            # prologue: gates + sigmoids for step 0
            g0_cur, S_cur, SoT_cur = emit_g0_sig(0)
            So1T_prev = None
            thc1_prev = None

            for t in range(t_steps):
                S, SoT = S_cur, SoT_cur
                Sv = S.rearrange("p (c f) -> p c f", c=2)
                An = A_q[t + 1]

                # ---- GP: forget product; DVE: L0 c update (bf16 4x)
                c0v = c0.rearrange("p (c f) -> p c f", c=2)
                m2 = ew.tile([128, 200], BF16, tag="m2")
                m2v = m2.rearrange("p (c f) -> p c f", c=2)
                nc.gpsimd.tensor_tensor(m2v, Sv[:, :, 100:200], c0v, Alu.mult)

                ts = ew.tile([128, 200], BF16, tag="ts")
                tsv = ts.rearrange("p (c f) -> p c f", c=2)
                nc.vector.tensor_scalar(tsv, Sv[:, :, 200:300], 2.0, 1.0,
                                        Alu.mult, Alu.subtract)
                m1 = ew.tile([128, 200], BF16, tag="m1")
                m1v = m1.rearrange("p (c f) -> p c f", c=2)
                nc.vector.tensor_tensor(m1v, Sv[:, :, 0:100], tsv, Alu.mult)
                c0n = ew.tile([128, 200], BF16, tag="c0")
                nc.vector.tensor_tensor(c0n, m1, m2, Alu.add)

                # ---- DVE: finish PREVIOUS step's h1 (frees B for g1(t))
                Bn = bpool.tile([50, 256], BF16, tag="B")
                if thc1_prev is not None:
                    nc.vector.tensor_tensor(Bn, So1T_prev, thc1_prev, Alu.mult)
                else:
                    nc.vector.memset(Bn[:, :], 0.0)
                Btile = Bn

                # ---- PE: heater + c0 transpose
                for k in range(HEAT1):
                    nc.tensor.matmul(
                        g0_cur[0:1, 300 : 300 + HEAT_COLS],
                        ident[0:1, 0:1],
                        cb[0:1, 0:HEAT_COLS],
                        start=True, stop=True,
                    )
                pc = tpool.tile([100, 256], BF16, tag="pc")
                for c in range(NCH):
                    nc.tensor.transpose(
                        pc[:, c * 128 : (c + 1) * 128],
                        c0n[:, c * 100 : (c + 1) * 100],
                        ident,
                    )

                # ---- ACT tanh; DVE writes h0(t) transposed into A(t+1)
                thc = ew.tile([100, 256], BF16, tag="thc")
                nc.scalar.activation(thc, pc, Act.Tanh)
                nc.vector.tensor_tensor(An[0:100, :], SoT, thc, Alu.mult)

                # ---- PE: g0(t+1)/oT(t+1) + their sigmoids (pipelined ahead)
                if t + 1 < t_steps:
                    g0_cur, S_cur, SoT_cur = emit_g0_sig(t + 1)

                # ---- PE: g1(t) from A(t+1) rows 0:101 (= h0(t), ones)
                g1 = g1pool.tile([128, 512], F32, tag="g1")
                for c in range(NCH):
                    nc.tensor.matmul(
                        g1[:, c * 256 : c * 256 + 150],
                        An[0:101, c * 128 : (c + 1) * 128],
                        wih1[:, 0:150],
                        start=True, stop=False,
                    )
                    nc.tensor.matmul(
                        g1[:, c * 256 : c * 256 + 150],
                        Btile[:, c * 128 : (c + 1) * 128],
                        whh1[:, 0:150],
                        start=False, stop=True,
                    )
                po1 = opool.tile([50, 256], F32, tag="po1")
                nc.tensor.matmul(po1, wih1[:, 150:200], An[0:101, :],
                                 start=True, stop=False)
                nc.tensor.matmul(po1, whh1[:, 150:200], Btile,
                                 start=False, stop=True)

                # ---- ACT: L1 sigmoids
                g1v = g1.rearrange("p (c f) -> p c f", c=2)   # [128, 2, 256]
                S1 = ew.tile([128, 300], BF16, tag="S1")
                S1v = S1.rearrange("p (c f) -> p c f", c=2)   # [128, 2, 150]
                nc.scalar.activation(S1v, g1v[:, :, 0:150], Act.Sigmoid)
                So1T = ew.tile([50, 256], BF16, tag="So1T")
                nc.scalar.activation(So1T, po1, Act.Sigmoid)

                # ---- GP: L1 forget product; DVE: L1 c update
                c1v = c1.rearrange("p (c f) -> p c f", c=2)
                m3 = ew.tile([128, 100], BF16, tag="m3")
                m3v = m3.rearrange("p (c f) -> p c f", c=2)
                nc.gpsimd.tensor_tensor(m3v, S1v[:, :, 50:100], c1v, Alu.mult)
                ts1 = ew.tile([128, 100], BF16, tag="ts1")
                ts1v = ts1.rearrange("p (c f) -> p c f", c=2)
                nc.vector.tensor_scalar(ts1v, S1v[:, :, 100:150], 2.0, 1.0,
                                        Alu.mult, Alu.subtract)
                m4 = ew.tile([128, 100], BF16, tag="m4")
                m4v = m4.rearrange("p (c f) -> p c f", c=2)
                nc.vector.tensor_tensor(m4v, S1v[:, :, 0:50], ts1v, Alu.mult)
                c1n = ew.tile([128, 100], BF16, tag="c1")
                nc.vector.tensor_tensor(c1n, m4, m3, Alu.add)

                # ---- PE: c1 transpose + heater; ACT: tanh
                pc1 = tpool.tile([50, 256], BF16, tag="pc1")
                for c in range(NCH):
                    nc.tensor.transpose(
                        pc1[:, c * 128 : (c + 1) * 128],
                        c1n[:, c * 50 : (c + 1) * 50],
                        ident,
                    )
                for k in range(HEAT2):
                    nc.tensor.matmul(
                        g0_cur[0:1, 512 + 300 : 512 + 300 + HEAT_COLS],
                        ident[0:1, 0:1],
                        cb[0:1, 0:HEAT_COLS],
                        start=True, stop=True,
                    )
                thc1 = ew.tile([50, 256], BF16, tag="thc1")
                nc.scalar.activation(thc1, pc1, Act.Tanh)
                So1T_prev, thc1_prev = So1T, thc1

                # ---- prefetch x for step t+1+PREFETCH
                if t + 1 + PREFETCH <= t_steps:
                    a = apool.tile([117, 256], BF16, tag="A")
                    nc.sync.dma_start(a[100:117, :], xt_d[t + 1 + PREFETCH])
                    A_q.append(a)

                c0, c1 = c0n, c1n

            # ---- epilogue: final h1, then y = h1 @ W_fc.T + b_fc
            Blast = bpool.tile([50, 256], BF16, tag="B")
            nc.vector.tensor_tensor(Blast, So1T_prev, thc1_prev, Alu.mult)
            Btile = Blast
            fin = ew.tile([51, 256], BF16, tag="fin")
            nc.vector.memset(fin[:, :], 1.0)
            nc.vector.tensor_copy(fin[0:50, :], Btile)
            yp = g1pool.tile([1, 256], F32, tag="yp")
            nc.tensor.matmul(yp, wfcb, fin, start=True, stop=True)
            ysb = ew.tile([1, 256], F32, tag="ysb")
            nc.scalar.copy(ysb, yp)
            nc.sync.dma_start(y_d, ysb)
    return nc


# ---------------------------------------------------------------- host prep


def _pack_gates(w, h):
    """[4h, d] torch-order (i,f,g,o) -> [d, 4h] columns (i, f, 2g, o)."""
    wt = np.asarray(w, np.float32).T if w.ndim == 2 else np.asarray(w, np.float32)[None, :]
    i, f, g, o = wt[:, 0:h], wt[:, h:2*h], wt[:, 2*h:3*h], wt[:, 3*h:4*h]
    return np.concatenate([i, f, 2.0 * g, o], axis=1)


def prep_weights(W_ih0, W_hh0, b_ih0, b_hh0, W_ih1, W_hh1, b_ih1, b_hh1, W_fc, b_fc):
    cb = np.zeros((128, 929), np.float32)
    cb[0:100, 0:400] = _pack_gates(W_hh0, H1)
    cb[100, 0:400] = _pack_gates(np.asarray(b_ih0) + np.asarray(b_hh0), H1)[0]
    cb[101:117, 0:400] = _pack_gates(W_ih0, H1)
    cb[0:100, 400:600] = _pack_gates(W_ih1, H2)
    cb[100, 400:600] = _pack_gates(np.asarray(b_ih1) + np.asarray(b_hh1), H2)[0]
    cb[0:50, 600:800] = _pack_gates(W_hh1, H2)
    cb[0:50, 800] = np.asarray(W_fc, np.float32)[0]
    cb[50, 800] = float(np.asarray(b_fc).reshape(-1)[0])
    cb[:, 801:929] = np.eye(128, dtype=np.float32)
    return cb.astype(BF)


def prep_x_core(x_core, t_steps):
    """x_core [256, T, 16] -> bf16 [T+1, 17, 256] with ones row at index 0."""
    xt = np.empty((t_steps + 1, 17, 256), BF)
    xt[:, 0, :] = np.asarray(1.0, BF)
    xt[:t_steps, 1:17, :] = np.asarray(x_core, np.float32).transpose(1, 2, 0).astype(BF)
    xt[t_steps, 1:17, :] = np.asarray(0.0, BF)
    return xt


_RUNNER_CACHE = {}


def _get_runner(t_steps):
    if t_steps in _RUNNER_CACHE:
        return _RUNNER_CACHE[t_steps]

    import jax
    from jax.experimental.shard_map import shard_map
    from jax.sharding import Mesh, NamedSharding, PartitionSpec

    from concourse import bass2jax

    bass2jax.install_neuronx_cc_hook()
    nc = build_nc(t_steps)
    if not nc.is_finalized():
        nc.finalize()
    global _LAST_NC
    _LAST_NC = nc

    partition_name = (
        nc.partition_id_tensor.name if nc.partition_id_tensor else None
    )
    in_names = []
    out_names = []
    out_avals = []
    zero_outs = []
    for alloc in nc.m.functions[0].allocations:
        if not isinstance(alloc, mybir.MemoryLocationSet):
            continue
        name = alloc.memorylocations[0].name
        if alloc.kind == "ExternalInput":
            if name == partition_name:
                continue
            in_names.append(name)
        elif alloc.kind == "ExternalOutput":
            out_names.append(name)
            shape = tuple(alloc.tensor_shape)
            dtype = mybir.dt.np(alloc.dtype)
            out_avals.append(jax.core.ShapedArray(shape, dtype))
            zero_outs.append(np.zeros(shape, dtype))
    n_params = len(in_names)
    all_in_names = in_names + out_names
    if partition_name is not None:
        all_in_names = all_in_names + [partition_name]

    def _body(*args):
        operands = list(args)
        if partition_name is not None:
            operands.append(bass2jax.partition_id_tensor())
        outs = bass2jax._bass_exec_p.bind(
            *operands,
            out_avals=tuple(out_avals),
            in_names=tuple(all_in_names),
            out_names=tuple(out_names),
            lowering_input_output_aliases=(),
            sim_require_finite=True,
            sim_require_nnan=True,
            nc=nc,
        )
        return tuple(outs)

    devices = jax.devices()[:N_CORES]
    mesh = Mesh(np.asarray(devices), ("core",))
    spec = PartitionSpec("core")
    in_specs = (spec,) * (n_params + len(out_names))
    out_specs = (spec,) * len(out_names)
    sharded = jax.jit(
        shard_map(_body, mesh=mesh, in_specs=in_specs, out_specs=out_specs,
                  check_rep=False),
        keep_unused=True,
    )
    sharding = NamedSharding(mesh, spec)

    def run(concat_inputs, n_bench=0):
        import time as _time

        args = [jax.device_put(concat_inputs[n], sharding) for n in in_names]
        args += [jax.device_put(
            np.zeros((N_CORES * z.shape[0], *z.shape[1:]), z.dtype), sharding)
            for z in zero_outs]
        outs = jax.block_until_ready(sharded(*args))
        bench_ns = None
        if n_bench:
            times = []
            for _ in range(n_bench):
                t0 = _time.perf_counter()
                jax.block_until_ready(sharded(*args))
                times.append(_time.perf_counter() - t0)
            bench_ns = int(min(times) * 1e9)
        y = np.asarray(outs[out_names.index("y")])
        return y, bench_ns

    _RUNNER_CACHE[t_steps] = run
    return run


def make_inputs(x, W_ih0, W_hh0, b_ih0, b_hh0, W_ih1, W_hh1, b_ih1, b_hh1,
                W_fc, b_fc):
    x = np.asarray(x, dtype=np.float32)
    t_steps = x.shape[1]
    cb = prep_weights(
        W_ih0, W_hh0, b_ih0, b_hh0, W_ih1, W_hh1, b_ih1, b_hh1, W_fc, b_fc
    )
    xt_all = np.empty((N_CORES * (t_steps + 1), 17, 256), BF)
    a0_all = np.zeros((N_CORES * 117, 256), BF)
    for core in range(N_CORES):
        xc = x[core * B_LOCAL : (core + 1) * B_LOCAL]
        xt = prep_x_core(xc, t_steps)
        xt_all[core * (t_steps + 1) : (core + 1) * (t_steps + 1)] = xt
        a0_all[core * 117 + 100 : (core + 1) * 117] = xt[0]
    reps = lambda a: np.concatenate([a] * N_CORES, axis=0)
    return t_steps, {
        "xt": xt_all,
        "cblob": reps(cb),
        "a0": a0_all,
    }


def kernel(x, W_ih0, W_hh0, b_ih0, b_hh0, W_ih1, W_hh1, b_ih1, b_hh1, W_fc, b_fc,
           n_bench=0):
    global LAST_EXEC_NS
    t_steps, concat_inputs = make_inputs(
        x, W_ih0, W_hh0, b_ih0, b_hh0, W_ih1, W_hh1, b_ih1, b_hh1, W_fc, b_fc
    )
    run = _get_runner(t_steps)
    y, bench_ns = run(concat_inputs, n_bench=n_bench)
    if bench_ns is not None:
        LAST_EXEC_NS = bench_ns
    return np.ascontiguousarray(y.reshape(-1)[:, None]).astype(np.float32)
